# revision 13
# baseline (speedup 1.0000x reference)
"""Transformer block (QKV + causal MHA + proj + GELU-FF, residual) on 8 NeuronCores.

Sharding: DP over batch (2 groups of 4 cores) x TP over heads / FF-inner within
each group. Identical SPMD program on all cores; per-core differences are input
slices only.

Design (from NTFF trace analysis):
- All matmul operands bf16 (f32 PSUM accumulate); end-to-end error ~3.5e-3.
- x loaded once to SBUF (bf16, 8MB), reused by qkv and ff1; wqk/wv/w1
  streamed exactly once, w2 per token-chunk.
- Phase order P1a(qk) P1b(v) P2(attn) P3a(ff1) P3b(proj+ff2+RS) with FF in
  two half-T passes so h fits SBUF; exp/gelu ACT tables load once each.
- exp batched over kt pairs ([128,2,512] PSUM groups); gelu over chunk pairs.
- softmax denominator: e-pairs pre-added on DVE, halving the ones-matmul
  count; 1/sum via reciprocal_approx_fast; broadcast via ones-row matmul.
- ReduceScatter in bf16 with 4-deep DRAM buffers, chunked per 512 tokens to
  overlap the collective with compute.
Host adds x + b_ff2 (residual) during unshard; output returned bf16.
"""
import numpy as np
import ml_dtypes

import concourse.bass as bass
import concourse.mybir as mybir
import concourse.tile as tile
from concourse import bacc
from concourse import bass_utils

B, T, C = 2, 2048, 2048
H, HD = 16, 128
F = 8192
NCORES = 8
TPG = 4                  # cores per batch group
HPC = H // TPG           # heads per core
QC = 4                   # token chunks per batch
TCH = T // QC            # 512
KT = C // 128            # 16
FPC = F // TPG           # 2048 ff rows per core
FT = FPC // 128          # 16
COT = C // 128           # 16
SM_SCALE = 1.0 / float(np.sqrt(HD))
NEG = -60000.0

f32r = mybir.dt.float32r
f32 = mybir.dt.float32
bf16 = mybir.dt.bfloat16

_CACHED_NC = None


def build_nc(rep=1):
    nc = bacc.Bacc("TRN2", target_bir_lowering=False, debug=False,
                   num_devices=NCORES)
    xb_t = nc.dram_tensor("xb", [128, KT, T], bf16, kind="ExternalInput").ap()
    wqk_t = nc.dram_tensor("wqk", [128, 2 * HPC, KT, 128], bf16,
                           kind="ExternalInput").ap()
    wv_t = nc.dram_tensor("wv", [128, KT, HPC * HD], bf16,
                          kind="ExternalInput").ap()
    wp_t = nc.dram_tensor("wp", [128, TPG, C], bf16, kind="ExternalInput").ap()
    w1_t = nc.dram_tensor("w1", [128, FT, KT, 128], bf16,
                          kind="ExternalInput").ap()
    b1_t = nc.dram_tensor("b1", [128, FT], f32, kind="ExternalInput").ap()
    w2_t = nc.dram_tensor("w2", [128, COT, FT, 128], bf16,
                          kind="ExternalInput").ap()
    out_t = nc.dram_tensor("outp", [C // TPG, T], bf16,
                           kind="ExternalOutput").ap()

    Exp = mybir.ActivationFunctionType.Exp
    Gelu = mybir.ActivationFunctionType.Gelu

    with tile.TileContext(nc) as tc:
        with tc.tile_pool(name="cst", bufs=1) as cst, \
             tc.tile_pool(name="ps", bufs=1, space="PSUM") as ps, \
             tc.tile_pool(name="dram", bufs=1, space="DRAM") as dram:

            ones_col = cst.tile([128, 1], bf16, name="ones_col", tag="oc")
            nc.gpsimd.memset(ones_col[:], 1.0)
            ones_row = cst.tile([1, 128], bf16, name="ones_row", tag="or")
            nc.gpsimd.memset(ones_row[:], 1.0)
            masks = cst.tile([128, QC, TCH], f32, name="masks", tag="mask")
            nc.gpsimd.memset(masks[:], 0.0)
            for d in range(QC):
                nc.gpsimd.affine_select(
                    out=masks[:, d, :], in_=masks[:, d, :],
                    compare_op=mybir.AluOpType.is_ge,
                    fill=NEG, base=-d * 128,
                    pattern=[[1, TCH]], channel_multiplier=-1,
                )
            b1_sb = cst.tile([128, FT], f32, name="b1_sb", tag="b1")
            nc.sync.dma_start(b1_sb[:], b1_t)

            for _rep in range(rep):
              # attnT written by P2, read by P3b; outermost per-rep pool
              with tc.tile_pool(name="pt", bufs=1) as pT:
                attnT = pT.tile([128, HPC, QC, TCH], bf16, name="attnT",
                                tag="attnT")
                # x resident for P1a/P1b and ff1
                with tc.tile_pool(name="px", bufs=1) as pX:
                    x_sb = pX.tile([128, KT, T], bf16, name="x_sb", tag="x")
                    nc.sync.dma_start(x_sb[:], xb_t)

                    # qk / v, alive through P2
                    with tc.tile_pool(name="pa", bufs=1) as pA:
                        qk_sb = pA.tile([128, 2 * HPC, T], bf16,
                                        name="qk_sb", tag="qk")
                        v_sb = pA.tile([128, T // 128, HPC * HD], bf16,
                                       name="v_sb", tag="v")
                        wv_sb = pA.tile([128, KT, HPC * HD], bf16,
                                        name="wv_sb", tag="wv", bufs=1)
                        nc.sync.dma_start(wv_sb[:], wv_t)

                        # P1a: qT/kT = w_qk^T @ x (feature-major out)
                        with tc.tile_pool(name="p1aw", bufs=1) as p1aw:
                            for ft in range(2 * HPC):
                                wqkt = p1aw.tile([128, KT, 128], bf16,
                                                 name="wqkt", tag="wqkt",
                                                 bufs=3)
                                nc.sync.dma_start(wqkt[:], wqk_t[:, ft])
                                for g in range(2):
                                    pt = ps.tile([128, 2, TCH], f32,
                                                 name="pmm", tag="pmm",
                                                 bufs=2)
                                    for hf in range(2):
                                        c = 2 * g + hf
                                        for k in range(KT):
                                            nc.tensor.matmul(
                                                pt[:, hf, :], wqkt[:, k, :],
                                                x_sb[:, k,
                                                     c * TCH:(c + 1) * TCH],
                                                start=(k == 0),
                                                stop=(k == KT - 1))
                                    for hf in range(2):
                                        c = 2 * g + hf
                                        nc.vector.tensor_copy(
                                            qk_sb[:, ft,
                                                  c * TCH:(c + 1) * TCH],
                                            pt[:, hf, :])

                        # P1b: v = x @ w_v (token-major out)
                        with tc.tile_pool(name="p1bw", bufs=1) as p1bw:
                            for mg in range(T // 256):
                                pt = ps.tile([128, 2, TCH], f32, name="pmm",
                                             tag="pmm", bufs=2)
                                for hf in range(2):
                                    m = 2 * mg + hf
                                    for k in range(KT):
                                        nc.tensor.matmul(
                                            pt[:, hf, :],
                                            x_sb[:, k, m * 128:(m + 1) * 128],
                                            wv_sb[:, k, :],
                                            start=(k == 0),
                                            stop=(k == KT - 1))
                                for hf in range(2):
                                    m = 2 * mg + hf
                                    nc.vector.tensor_copy(v_sb[:, m, :],
                                                          pt[:, hf, :])

                        # P2: causal attention (c outer, h inner)
                        with tc.tile_pool(name="p2w", bufs=1) as p2w:
                            for c in range(QC):
                                for h in range(HPC):
                                    nkt = 4 * c + 4
                                    ngrp = nkt // 2
                                    po = ps.tile([128, TCH], f32, name="po",
                                                 tag="po", bufs=2)
                                    psums = ps.tile([1, TCH], f32,
                                                    name="psums", tag="ps1",
                                                    bufs=2)
                                    for g in range(ngrp):
                                        pscore = ps.tile([128, 2, TCH], f32,
                                                         name="pmm",
                                                         tag="pmm", bufs=2)
                                        for t2 in range(2):
                                            kt = 2 * g + t2
                                            nc.tensor.matmul(
                                                pscore[:, t2, :],
                                                qk_sb[:, HPC + h,
                                                      kt * 128:
                                                      (kt + 1) * 128],
                                                qk_sb[:, h,
                                                      c * TCH:(c + 1) * TCH],
                                                start=True, stop=True)
                                        e2 = p2w.tile([128, 2, TCH], bf16,
                                                      name="e2", tag="e",
                                                      bufs=3)
                                        if 2 * g >= 4 * c:
                                            d0 = 2 * g - 4 * c
                                            ms = p2w.tile([128, 2, TCH],
                                                          f32, name="ms",
                                                          tag="ms", bufs=2)
                                            nc.vector.tensor_add(
                                                ms[:], pscore[:],
                                                masks[:, d0:d0 + 2, :])
                                            nc.scalar.activation(
                                                e2[:], ms[:], Exp,
                                                scale=SM_SCALE)
                                        else:
                                            nc.scalar.activation(
                                                e2[:], pscore[:], Exp,
                                                scale=SM_SCALE)
                                        # denominator: pre-add the pair on
                                        # DVE, one ones-matmul per pair
                                        epair = p2w.tile([128, TCH], bf16,
                                                         name="epair",
                                                         tag="ep", bufs=3)
                                        nc.vector.tensor_add(
                                            epair[:], e2[:, 0, :],
                                            e2[:, 1, :])
                                        nc.tensor.matmul(
                                            psums[:], ones_col[:], epair[:],
                                            start=(g == 0),
                                            stop=(g == ngrp - 1))
                                        for t2 in range(2):
                                            kt = 2 * g + t2
                                            nc.tensor.matmul(
                                                po[:],
                                                v_sb[:, kt,
                                                     h * HD:(h + 1) * HD],
                                                e2[:, t2, :],
                                                start=(kt == 0),
                                                stop=(kt == nkt - 1))
                                    recip = p2w.tile([1, TCH], f32,
                                                     name="recip",
                                                     tag="recip", bufs=2)
                                    nc.vector.reciprocal_approx_fast(
                                        recip[:], psums[:])
                                    recip_b = p2w.tile([1, TCH], bf16,
                                                       name="recip_b",
                                                       tag="recipb", bufs=2)
                                    nc.vector.tensor_copy(recip_b[:],
                                                          recip[:])
                                    pbc = ps.tile([128, TCH], f32,
                                                  name="pbc", tag="ps1",
                                                  bufs=2)
                                    nc.tensor.matmul(pbc[:], ones_row[:],
                                                     recip_b[:], start=True,
                                                     stop=True)
                                    bc_sb = p2w.tile([128, TCH], f32,
                                                     name="bc_sb", tag="bc",
                                                     bufs=2)
                                    nc.vector.tensor_copy(bc_sb[:], pbc[:])
                                    nc.vector.tensor_mul(
                                        attnT[:, h, c, :], po[:], bc_sb[:])

                    # FF + proj in two half-T passes (h half-resident to fit
                    # SBUF); gelu passes are adjacent so the exp/gelu ACT
                    # tables load once each per rep.
                    with tc.tile_pool(name="p3w", bufs=1) as p3w:
                        wp_sb = p3w.tile([128, TPG, C], bf16, name="wp_sb",
                                         tag="wp", bufs=1)
                        nc.sync.dma_start(wp_sb[:], wp_t)
                        for hh in range(2):
                            h_sb = p3w.tile([128, FT, 2, TCH], bf16,
                                            name="h_sb", tag="h", bufs=1)
                            # P3a: h = gelu(x @ w1 + b1) for chunks 2hh,2hh+1
                            for f in range(FT):
                                w1t = p3w.tile([128, KT, 128], bf16,
                                               name="w1t", tag="w1t", bufs=3)
                                nc.sync.dma_start(w1t[:], w1_t[:, f])
                                pt = ps.tile([128, 2, TCH], f32, name="pmm",
                                             tag="pmm", bufs=2)
                                for cc in range(2):
                                    c = 2 * hh + cc
                                    for k in range(KT):
                                        nc.tensor.matmul(
                                            pt[:, cc, :], w1t[:, k, :],
                                            x_sb[:, k,
                                                 c * TCH:(c + 1) * TCH],
                                            start=(k == 0),
                                            stop=(k == KT - 1))
                                nc.scalar.activation(
                                    h_sb[:, f, :, :], pt[:], Gelu,
                                    bias=b1_sb[:, f:f + 1], scale=1.0)
                            # P3b: out = wp^T @ attnT + w2^T @ h; chunked RS
                            for cc in range(2):
                                c = 2 * hh + cc
                                rs_in = dram.tile([COT * 128, TCH], bf16,
                                                  name="rs_in", tag="rsi",
                                                  bufs=4)
                                for co in range(COT):
                                    w2t = p3w.tile([128, FT, 128], bf16,
                                                   name="w2t", tag="w2t",
                                                   bufs=3)
                                    nc.sync.dma_start(w2t[:], w2_t[:, co])
                                    pout = ps.tile([128, TCH], f32,
                                                   name="pout", tag="po",
                                                   bufs=2)
                                    for k4 in range(TPG):
                                        nc.tensor.matmul(
                                            pout[:],
                                            wp_sb[:, k4,
                                                  co * 128:(co + 1) * 128],
                                            attnT[:, k4, c, :],
                                            start=(k4 == 0), stop=False)
                                    for ftile in range(FT):
                                        nc.tensor.matmul(
                                            pout[:], w2t[:, ftile, :],
                                            h_sb[:, ftile, cc, :],
                                            start=False,
                                            stop=(ftile == FT - 1))
                                    o_sb = p3w.tile([128, TCH], bf16,
                                                    name="o_sb", tag="o",
                                                    bufs=4)
                                    nc.vector.tensor_copy(o_sb[:], pout[:])
                                    nc.sync.dma_start(
                                        rs_in[co * 128:(co + 1) * 128, :],
                                        o_sb[:])
                                rs_out = dram.tile([(COT * 128) // TPG, TCH],
                                                   bf16, name="rs_out",
                                                   tag="rso", bufs=4)
                                nc.gpsimd.collective_compute(
                                    "ReduceScatter", mybir.AluOpType.add,
                                    replica_groups=[[0, 1, 2, 3],
                                                    [4, 5, 6, 7]],
                                    ins=[rs_in.opt()], outs=[rs_out.opt()])
                                nc.sync.dma_start(
                                    out_t[:, c * TCH:(c + 1) * TCH],
                                    rs_out[:])

    nc.compile()
    return nc


def _ptile(a, kt=None):
    """[kt*128, X] row-major -> [128, kt, X] partition-tiled contiguous."""
    rows = a.shape[0]
    kt = rows // 128 if kt is None else kt
    return np.ascontiguousarray(
        a.reshape(kt, 128, *a.shape[1:]).swapaxes(0, 1))


def make_in_maps(x, w_qkv, w_proj, w_ff1, b_ff1, w_ff2):
    in_maps = []
    bf = ml_dtypes.bfloat16
    for r in range(NCORES):
        b, hg = r // TPG, r % TPG
        q_cols = w_qkv[:, hg * 512:(hg + 1) * 512]
        k_cols = w_qkv[:, C + hg * 512:C + (hg + 1) * 512]
        v_cols = w_qkv[:, 2 * C + hg * 512:2 * C + (hg + 1) * 512]
        xT = np.ascontiguousarray(x[b].T)
        wqk = np.concatenate([q_cols, k_cols], axis=1)          # [C, 1024]
        # [128, ft, kt, 128]: wqk[kt*128+p, ft*128+j]
        wqk4 = wqk.reshape(KT, 128, 2 * HPC, 128).transpose(1, 2, 0, 3)
        w1 = w_ff1[:, hg * FPC:(hg + 1) * FPC]                  # [C, 2048]
        w14 = w1.reshape(KT, 128, FT, 128).transpose(1, 2, 0, 3)
        w2 = w_ff2[hg * FPC:(hg + 1) * FPC, :]                  # [2048, C]
        w24 = w2.reshape(FT, 128, COT, 128).transpose(1, 2, 0, 3)
        in_maps.append({
            "xb": _ptile(xT).astype(bf),
            "wqk": np.ascontiguousarray(wqk4).astype(bf),
            "wv": _ptile(v_cols).astype(bf),
            "wp": _ptile(w_proj[hg * 512:(hg + 1) * 512, :]).astype(bf),
            "w1": np.ascontiguousarray(w14).astype(bf),
            "b1": np.ascontiguousarray(
                b_ff1[hg * FPC:(hg + 1) * FPC].reshape(FT, 128).T),
            "w2": np.ascontiguousarray(w24).astype(bf),
        })
    return in_maps


def assemble(results, x, b_ff2):
    out = np.empty((B, T, C), np.float32)
    for r in range(NCORES):
        b, idx = r // TPG, r % TPG
        out[b, :, idx * 512:(idx + 1) * 512] = \
            results[r]["outp"].astype(np.float32).T
    out += x + b_ff2
    return out


def kernel(x, w_qkv, w_proj, w_ff1, b_ff1, w_ff2, b_ff2):
    global _CACHED_NC
    x = np.asarray(x, np.float32)
    if _CACHED_NC is None:
        _CACHED_NC = build_nc()
    in_maps = make_in_maps(x, np.asarray(w_qkv, np.float32),
                           np.asarray(w_proj, np.float32),
                           np.asarray(w_ff1, np.float32),
                           np.asarray(b_ff1, np.float32),
                           np.asarray(w_ff2, np.float32))
    res = bass_utils.run_bass_kernel_spmd(_CACHED_NC, in_maps,
                                          core_ids=list(range(NCORES)))
    return assemble(res.results, x, np.asarray(b_ff2, np.float32))


# revision 14
# speedup vs baseline: 1.2120x; 1.2120x over previous
"""Transformer block (QKV + causal MHA + proj + GELU-FF, residual) on 8 NeuronCores.

Sharding: DP over batch (2 groups of 4 cores) x TP over heads / FF-inner within
each group. Identical SPMD program on all cores; per-core differences are input
slices only.

Design (from NTFF trace analysis):
- All matmul operands bf16 (f32 PSUM accumulate); end-to-end error ~3.5e-3.
- x loaded once to SBUF (bf16, 8MB), reused by qkv and ff1; wqk/wv/w1
  streamed exactly once, w2 per token-chunk.
- Phase order P1a(qk) P1b(v) P2(attn) P3a(ff1) P3b(proj+ff2+RS) with FF in
  two half-T passes so h fits SBUF; exp/gelu ACT tables load once each.
- exp batched over kt pairs ([128,2,512] PSUM groups); gelu over chunk pairs.
- softmax denominator: e-pairs pre-added on DVE, halving the ones-matmul
  count; 1/sum via reciprocal_approx_fast; broadcast via ones-row matmul.
- ReduceScatter in bf16 with 4-deep DRAM buffers, chunked per 512 tokens to
  overlap the collective with compute.
Host adds x + b_ff2 (residual) during unshard; output returned bf16.
"""
import numpy as np
import ml_dtypes

import concourse.bass as bass
import concourse.mybir as mybir
import concourse.tile as tile
from concourse import bacc
from concourse import bass_utils

B, T, C = 2, 2048, 2048
H, HD = 16, 128
F = 8192
NCORES = 8
TPG = 4                  # cores per batch group
HPC = H // TPG           # heads per core
QC = 4                   # token chunks per batch
TCH = T // QC            # 512
KT = C // 128            # 16
FPC = F // TPG           # 2048 ff rows per core
FT = FPC // 128          # 16
COT = C // 128           # 16
SM_SCALE = 1.0 / float(np.sqrt(HD))
NEG = -60000.0

f32r = mybir.dt.float32r
f32 = mybir.dt.float32
bf16 = mybir.dt.bfloat16

_CACHED_NC = None


def build_nc(rep=1):
    nc = bacc.Bacc("TRN2", target_bir_lowering=False, debug=False,
                   num_devices=NCORES)
    xb_t = nc.dram_tensor("xb", [128, KT, T], bf16, kind="ExternalInput").ap()
    wqk_t = nc.dram_tensor("wqk", [128, 2 * HPC, KT, 128], bf16,
                           kind="ExternalInput").ap()
    wv_t = nc.dram_tensor("wv", [128, KT, HPC * HD], bf16,
                          kind="ExternalInput").ap()
    wp_t = nc.dram_tensor("wp", [128, TPG, C], bf16, kind="ExternalInput").ap()
    w1_t = nc.dram_tensor("w1", [128, FT, KT, 128], bf16,
                          kind="ExternalInput").ap()
    b1_t = nc.dram_tensor("b1", [128, FT], f32, kind="ExternalInput").ap()
    w2_t = nc.dram_tensor("w2", [128, COT, FT, 128], bf16,
                          kind="ExternalInput").ap()
    out_t = nc.dram_tensor("outp", [C // TPG, T], bf16,
                           kind="ExternalOutput").ap()

    Exp = mybir.ActivationFunctionType.Exp
    Gelu = mybir.ActivationFunctionType.Gelu

    with tile.TileContext(nc) as tc:
        with tc.tile_pool(name="cst", bufs=1) as cst, \
             tc.tile_pool(name="ps", bufs=1, space="PSUM") as ps, \
             tc.tile_pool(name="dram", bufs=1, space="DRAM") as dram:

            ones_col = cst.tile([128, 1], bf16, name="ones_col", tag="oc")
            nc.gpsimd.memset(ones_col[:], 1.0)
            ones_row = cst.tile([1, 128], bf16, name="ones_row", tag="or")
            nc.gpsimd.memset(ones_row[:], 1.0)
            masks = cst.tile([128, QC, TCH], f32, name="masks", tag="mask")
            nc.gpsimd.memset(masks[:], 0.0)
            for d in range(QC):
                nc.gpsimd.affine_select(
                    out=masks[:, d, :], in_=masks[:, d, :],
                    compare_op=mybir.AluOpType.is_ge,
                    fill=NEG, base=-d * 128,
                    pattern=[[1, TCH]], channel_multiplier=-1,
                )
            b1_sb = cst.tile([128, FT], f32, name="b1_sb", tag="b1")
            nc.sync.dma_start(b1_sb[:], b1_t)

            for _rep in range(rep):
              # attnT written by P2, read by P3b; outermost per-rep pool
              with tc.tile_pool(name="pt", bufs=1) as pT:
                attnT = pT.tile([128, HPC, QC, TCH], bf16, name="attnT",
                                tag="attnT")
                # x resident for P1a/P1b and ff1
                with tc.tile_pool(name="px", bufs=1) as pX:
                    x_sb = pX.tile([128, KT, T], bf16, name="x_sb", tag="x")
                    nc.sync.dma_start(x_sb[:], xb_t)

                    # qk / v, alive through P2
                    with tc.tile_pool(name="pa", bufs=1) as pA:
                        qk_sb = pA.tile([128, 2 * HPC, T], bf16,
                                        name="qk_sb", tag="qk")
                        v_sb = pA.tile([128, T // 128, HPC * HD], bf16,
                                       name="v_sb", tag="v")

                        # P1a: qT/kT = w_qk^T @ x (feature-major out)
                        with tc.tile_pool(name="p1aw", bufs=1) as p1aw:
                            for ft in range(2 * HPC):
                                wqkt = p1aw.tile([128, KT, 128], bf16,
                                                 name="wqkt", tag="wqkt",
                                                 bufs=3)
                                nc.sync.dma_start(wqkt[:], wqk_t[:, ft])
                                for g in range(2):
                                    pt = ps.tile([128, 2, TCH], f32,
                                                 name="pmm", tag="pmm",
                                                 bufs=2)
                                    for hf in range(2):
                                        c = 2 * g + hf
                                        for k in range(KT):
                                            nc.tensor.matmul(
                                                pt[:, hf, :], wqkt[:, k, :],
                                                x_sb[:, k,
                                                     c * TCH:(c + 1) * TCH],
                                                start=(k == 0),
                                                stop=(k == KT - 1))
                                    for hf in range(2):
                                        c = 2 * g + hf
                                        nc.vector.tensor_copy(
                                            qk_sb[:, ft,
                                                  c * TCH:(c + 1) * TCH],
                                            pt[:, hf, :])

                        # P1b: v = x @ w_v (token-major out)
                        with tc.tile_pool(name="p1bw", bufs=1) as p1bw:
                            wv_sb = p1bw.tile([128, KT, HPC * HD], bf16,
                                              name="wv_sb", tag="wv", bufs=1)
                            nc.sync.dma_start(wv_sb[:], wv_t)
                            for mg in range(T // 256):
                                pt = ps.tile([128, 2, TCH], f32, name="pmm",
                                             tag="pmm", bufs=2)
                                for hf in range(2):
                                    m = 2 * mg + hf
                                    for k in range(KT):
                                        nc.tensor.matmul(
                                            pt[:, hf, :],
                                            x_sb[:, k, m * 128:(m + 1) * 128],
                                            wv_sb[:, k, :],
                                            start=(k == 0),
                                            stop=(k == KT - 1))
                                for hf in range(2):
                                    m = 2 * mg + hf
                                    nc.vector.tensor_copy(v_sb[:, m, :],
                                                          pt[:, hf, :])

                        # P2: causal attention (c outer, h inner)
                        with tc.tile_pool(name="p2w", bufs=1) as p2w:
                            for c in range(QC):
                                for h in range(HPC):
                                    nkt = 4 * c + 4
                                    ngrp = nkt // 2
                                    po = ps.tile([128, TCH], f32, name="po",
                                                 tag="po", bufs=2)
                                    psums = ps.tile([1, TCH], f32,
                                                    name="psums", tag="ps1",
                                                    bufs=2)
                                    for g in range(ngrp):
                                        pscore = ps.tile([128, 2, TCH], f32,
                                                         name="pmm",
                                                         tag="pmm", bufs=2)
                                        for t2 in range(2):
                                            kt = 2 * g + t2
                                            nc.tensor.matmul(
                                                pscore[:, t2, :],
                                                qk_sb[:, HPC + h,
                                                      kt * 128:
                                                      (kt + 1) * 128],
                                                qk_sb[:, h,
                                                      c * TCH:(c + 1) * TCH],
                                                start=True, stop=True)
                                        e2 = p2w.tile([128, 2, TCH], bf16,
                                                      name="e2", tag="e",
                                                      bufs=3)
                                        if 2 * g >= 4 * c:
                                            d0 = 2 * g - 4 * c
                                            ms = p2w.tile([128, 2, TCH],
                                                          f32, name="ms",
                                                          tag="ms", bufs=2)
                                            nc.vector.tensor_add(
                                                ms[:], pscore[:],
                                                masks[:, d0:d0 + 2, :])
                                            nc.scalar.activation(
                                                e2[:], ms[:], Exp,
                                                scale=SM_SCALE)
                                        else:
                                            nc.scalar.activation(
                                                e2[:], pscore[:], Exp,
                                                scale=SM_SCALE)
                                        # denominator: pre-add the pair on
                                        # DVE, one ones-matmul per pair
                                        epair = p2w.tile([128, TCH], bf16,
                                                         name="epair",
                                                         tag="ep", bufs=3)
                                        nc.vector.tensor_add(
                                            epair[:], e2[:, 0, :],
                                            e2[:, 1, :])
                                        nc.tensor.matmul(
                                            psums[:], ones_col[:], epair[:],
                                            start=(g == 0),
                                            stop=(g == ngrp - 1))
                                        for t2 in range(2):
                                            kt = 2 * g + t2
                                            nc.tensor.matmul(
                                                po[:],
                                                v_sb[:, kt,
                                                     h * HD:(h + 1) * HD],
                                                e2[:, t2, :],
                                                start=(kt == 0),
                                                stop=(kt == nkt - 1))
                                    recip = p2w.tile([1, TCH], f32,
                                                     name="recip",
                                                     tag="recip", bufs=2)
                                    nc.vector.reciprocal_approx_fast(
                                        recip[:], psums[:])
                                    recip_b = p2w.tile([1, TCH], bf16,
                                                       name="recip_b",
                                                       tag="recipb", bufs=2)
                                    nc.vector.tensor_copy(recip_b[:],
                                                          recip[:])
                                    pbc = ps.tile([128, TCH], f32,
                                                  name="pbc", tag="ps1",
                                                  bufs=2)
                                    nc.tensor.matmul(pbc[:], ones_row[:],
                                                     recip_b[:], start=True,
                                                     stop=True)
                                    bc_sb = p2w.tile([128, TCH], f32,
                                                     name="bc_sb", tag="bc",
                                                     bufs=2)
                                    nc.vector.tensor_copy(bc_sb[:], pbc[:])
                                    nc.vector.tensor_mul(
                                        attnT[:, h, c, :], po[:], bc_sb[:])

                    # FF + proj in two half-T passes (h half-resident to fit
                    # SBUF); gelu passes are adjacent so the exp/gelu ACT
                    # tables load once each per rep.
                    with tc.tile_pool(name="p3w", bufs=1) as p3w:
                        wp_sb = p3w.tile([128, TPG, C], bf16, name="wp_sb",
                                         tag="wp", bufs=1)
                        nc.sync.dma_start(wp_sb[:], wp_t)
                        for hh in range(2):
                            h_sb = p3w.tile([128, FT, 2, TCH], bf16,
                                            name="h_sb", tag="h", bufs=1)
                            # P3a: h = gelu(x @ w1 + b1) for chunks 2hh,2hh+1
                            for f in range(FT):
                                w1t = p3w.tile([128, KT, 128], bf16,
                                               name="w1t", tag="w1t", bufs=3)
                                nc.sync.dma_start(w1t[:], w1_t[:, f])
                                pt = ps.tile([128, 2, TCH], f32, name="pmm",
                                             tag="pmm", bufs=2)
                                for cc in range(2):
                                    c = 2 * hh + cc
                                    for k in range(KT):
                                        nc.tensor.matmul(
                                            pt[:, cc, :], w1t[:, k, :],
                                            x_sb[:, k,
                                                 c * TCH:(c + 1) * TCH],
                                            start=(k == 0),
                                            stop=(k == KT - 1))
                                nc.scalar.activation(
                                    h_sb[:, f, :, :], pt[:], Gelu,
                                    bias=b1_sb[:, f:f + 1], scale=1.0)
                            # P3b: out = wp^T @ attnT + w2^T @ h; chunked RS
                            for cc in range(2):
                                c = 2 * hh + cc
                                rs_in = dram.tile([COT * 128, TCH], bf16,
                                                  name="rs_in", tag="rsi",
                                                  bufs=4)
                                for co in range(COT):
                                    w2t = p3w.tile([128, FT, 128], bf16,
                                                   name="w2t", tag="w2t",
                                                   bufs=3)
                                    nc.sync.dma_start(w2t[:], w2_t[:, co])
                                    pout = ps.tile([128, TCH], f32,
                                                   name="pout", tag="po",
                                                   bufs=2)
                                    for k4 in range(TPG):
                                        nc.tensor.matmul(
                                            pout[:],
                                            wp_sb[:, k4,
                                                  co * 128:(co + 1) * 128],
                                            attnT[:, k4, c, :],
                                            start=(k4 == 0), stop=False)
                                    for ftile in range(FT):
                                        nc.tensor.matmul(
                                            pout[:], w2t[:, ftile, :],
                                            h_sb[:, ftile, cc, :],
                                            start=False,
                                            stop=(ftile == FT - 1))
                                    o_sb = p3w.tile([128, TCH], bf16,
                                                    name="o_sb", tag="o",
                                                    bufs=4)
                                    nc.vector.tensor_copy(o_sb[:], pout[:])
                                    nc.sync.dma_start(
                                        rs_in[co * 128:(co + 1) * 128, :],
                                        o_sb[:])
                                rs_out = dram.tile([(COT * 128) // TPG, TCH],
                                                   bf16, name="rs_out",
                                                   tag="rso", bufs=4)
                                nc.gpsimd.collective_compute(
                                    "ReduceScatter", mybir.AluOpType.add,
                                    replica_groups=[[0, 1, 2, 3],
                                                    [4, 5, 6, 7]],
                                    ins=[rs_in.opt()], outs=[rs_out.opt()])
                                nc.sync.dma_start(
                                    out_t[:, c * TCH:(c + 1) * TCH],
                                    rs_out[:])

    nc.compile()
    return nc


def _ptile(a, kt=None):
    """[kt*128, X] row-major -> [128, kt, X] partition-tiled contiguous."""
    rows = a.shape[0]
    kt = rows // 128 if kt is None else kt
    return np.ascontiguousarray(
        a.reshape(kt, 128, *a.shape[1:]).swapaxes(0, 1))


def make_in_maps(x, w_qkv, w_proj, w_ff1, b_ff1, w_ff2):
    in_maps = []
    bf = ml_dtypes.bfloat16
    for r in range(NCORES):
        b, hg = r // TPG, r % TPG
        q_cols = w_qkv[:, hg * 512:(hg + 1) * 512]
        k_cols = w_qkv[:, C + hg * 512:C + (hg + 1) * 512]
        v_cols = w_qkv[:, 2 * C + hg * 512:2 * C + (hg + 1) * 512]
        xT = np.ascontiguousarray(x[b].T)
        wqk = np.concatenate([q_cols, k_cols], axis=1)          # [C, 1024]
        # [128, ft, kt, 128]: wqk[kt*128+p, ft*128+j]
        wqk4 = wqk.reshape(KT, 128, 2 * HPC, 128).transpose(1, 2, 0, 3)
        w1 = w_ff1[:, hg * FPC:(hg + 1) * FPC]                  # [C, 2048]
        w14 = w1.reshape(KT, 128, FT, 128).transpose(1, 2, 0, 3)
        w2 = w_ff2[hg * FPC:(hg + 1) * FPC, :]                  # [2048, C]
        w24 = w2.reshape(FT, 128, COT, 128).transpose(1, 2, 0, 3)
        in_maps.append({
            "xb": _ptile(xT).astype(bf),
            "wqk": np.ascontiguousarray(wqk4).astype(bf),
            "wv": _ptile(v_cols).astype(bf),
            "wp": _ptile(w_proj[hg * 512:(hg + 1) * 512, :]).astype(bf),
            "w1": np.ascontiguousarray(w14).astype(bf),
            "b1": np.ascontiguousarray(
                b_ff1[hg * FPC:(hg + 1) * FPC].reshape(FT, 128).T),
            "w2": np.ascontiguousarray(w24).astype(bf),
        })
    return in_maps


def assemble(results, x, b_ff2):
    out = np.empty((B, T, C), np.float32)
    for r in range(NCORES):
        b, idx = r // TPG, r % TPG
        out[b, :, idx * 512:(idx + 1) * 512] = \
            results[r]["outp"].astype(np.float32).T
    out += x + b_ff2
    return out


def kernel(x, w_qkv, w_proj, w_ff1, b_ff1, w_ff2, b_ff2):
    global _CACHED_NC
    x = np.asarray(x, np.float32)
    if _CACHED_NC is None:
        _CACHED_NC = build_nc()
    in_maps = make_in_maps(x, np.asarray(w_qkv, np.float32),
                           np.asarray(w_proj, np.float32),
                           np.asarray(w_ff1, np.float32),
                           np.asarray(b_ff1, np.float32),
                           np.asarray(w_ff2, np.float32))
    res = bass_utils.run_bass_kernel_spmd(_CACHED_NC, in_maps,
                                          core_ids=list(range(NCORES)))
    return assemble(res.results, x, np.asarray(b_ff2, np.float32))
